# revision 1
# baseline (speedup 1.0000x reference)
"""Distributed Trainium2 Bass kernel for the causal AttentionBlock problem.

Shapes (hardcoded): B=2, S=2048, D=2048, H=16, HD=128, softcap 50, RoPE.

Sharding: DP over batch (2 groups of 4 cores) x TP over heads (4 heads/core).
Each core computes, for its batch b = core//4 and heads 4*(core%4)..+4:
  qkv projection -> RoPE -> causal softcapped attention -> output projection
producing partial [512, D] row-blocks; slab-granular ReduceScatter(add) over
the 4-core group yields each core's rows of the final output (bf16; host
converts to f32 and reassembles).

Design notes (v2):
- Single fused emission pipeline: projections for s-chunk t+1 are interleaved
  with attention over q-chunk t and the output projection of q-chunk t-1, so
  the PE never starves (keeps the HAM clock-gate at K=8/8).
- Scores for two k-tiles are packed into one [128, 1024] 2-bank PSUM tile;
  tanh runs in-place on PSUM and exp writes bf16 to SBUF, halving ACT
  instruction count (the 352-cycle per-op overhead dominates small tiles).
- Causal masking is done post-exp with static mask tiles (full-width
  matmuls), attention row-sums via DVE accumulation + one ones-matmul per
  (chunk, head), reciprocal via the fast approx DVE op.
- RoPE is computed directly from the projection PSUM with cross-partition
  DVE reads (legal when one input is PSUM) - no SBUF swap DMAs.

Device-side compute dtype: bf16 matmul inputs / fp32 accumulation.
Host-pre-transposed inputs so every matmul contraction is on the partition
axis:
  xt   [128, 16, 2048]  xt[di,do,s] = x[s, do*128+di]            (bf16)
  wq   [128, 16, 4, 128] wq[di,do,h,f] = w_in[do*128+di, h, f]   (bf16)
  wk   same for f+128; wv [128,16,512] for f+256 (h-major free)  (bf16)
  wo   [128, 4, 2048]   wo[hd,h,m] = w_out[h, hd, m]             (bf16)
  cs2  [128, 2048]      rows 0:64 = cos.T, rows 64:128 = cos.T   (f32)
  sn2  [128, 2048]      rows 0:64 = -sin.T, rows 64:128 = sin.T  (f32)
  mka/mkb [128, 1024]   causal masks for diagonal k-tile pairs   (bf16)
"""

import math
import os
import sys

import numpy as np

sys.path.insert(0, "/opt/trn_rl_repo")

import ml_dtypes  # noqa: E402

import concourse.bass as bass  # noqa: E402
import concourse.mybir as mybir  # noqa: E402
import concourse.tile as tile  # noqa: E402
from concourse.tile import add_dep_helper  # noqa: E402
from concourse import bacc, bass_utils  # noqa: E402

B, S, D, H = 2, 2048, 2048, 16
HD = 128
HALF = 64
NH = 4          # heads per core
NCORES = 8
GROUPS = [[0, 1, 2, 3], [4, 5, 6, 7]]
SC = 512        # s-chunk (q-chunk size)
P = 128
DO = D // P     # 16 d-outer chunks
ST = S // P     # 16 s-tiles
QC = S // SC    # 4 q chunks
SCAP = 50.0
F32 = mybir.dt.float32
BF16 = mybir.dt.bfloat16
ADD = None      # filled after mybir import in build
SPLITS = (1, 1, 1, 1)   # ReduceScatter splits per q-chunk

_CACHE = {}
LAST_EXEC_NS = None
LAST_RESULT = None


def _install_profile_shim():
    """Provide antenv.axon_hooks (missing in this image) so
    run_bass_kernel_spmd(trace=True) can NTFF-profile via libaxon."""
    import contextlib
    import ctypes
    import types

    try:
        import antenv
    except ImportError:
        return
    if "antenv.axon_hooks" in sys.modules:
        return
    try:
        from antenv import axon_hooks  # noqa: F401
        return
    except ImportError:
        pass
    so_path = "/opt/axon/libaxon_pjrt.so"
    if not os.path.exists(so_path):
        return
    mod = types.ModuleType("antenv.axon_hooks")
    state = {"hook": None}
    mod.set_axon_ntff_profile_hook = lambda h: state.__setitem__("hook", h)
    mod.get_axon_ntff_profile_hook = lambda: state["hook"]
    sys.modules["antenv.axon_hooks"] = mod
    antenv.axon_hooks = mod

    lib = ctypes.CDLL(so_path)
    if not hasattr(lib, "axon_start_nrt_profile"):
        return
    lib.axon_start_nrt_profile.argtypes = [
        ctypes.POINTER(ctypes.c_int64), ctypes.c_size_t]
    lib.axon_start_nrt_profile.restype = ctypes.c_int64
    lib.axon_stop_nrt_profile.argtypes = [ctypes.c_char_p]
    lib.axon_stop_nrt_profile.restype = ctypes.c_int64

    @contextlib.contextmanager
    def _hook(output_dir, device_ids):
        import jax
        jax.devices()
        if device_ids:
            ids = (ctypes.c_int64 * len(device_ids))(*device_ids)
            rc = lib.axon_start_nrt_profile(ids, len(device_ids))
        else:
            rc = lib.axon_start_nrt_profile(None, 0)
        if rc != 0:
            raise RuntimeError(f"axon_start_nrt_profile rc={rc}")
        try:
            yield
        finally:
            n = lib.axon_stop_nrt_profile(str(output_dir).encode())
            print(f"profile: {n} file(s) written to {output_dir}",
                  file=sys.stderr)

    mod.set_axon_ntff_profile_hook(_hook)


def _weave(*seqs):
    """Proportionally interleave several lists."""
    import heapq
    seqs = [s for s in seqs if s]
    h = [(0.5 / len(s), i, 0) for i, s in enumerate(seqs)]
    heapq.heapify(h)
    out = []
    while h:
        _, i, j = heapq.heappop(h)
        out.append(seqs[i][j])
        if j + 1 < len(seqs[i]):
            heapq.heappush(h, ((j + 1.5) / len(seqs[i]), i, j + 1))
    return out


def build_nc():
    nc = bacc.Bacc("TRN2", target_bir_lowering=False, debug=False,
                   num_devices=NCORES)

    xt_d = nc.dram_tensor("xt", [QC, P, DO * SC], BF16, kind="ExternalInput")
    wq_d = nc.dram_tensor("wq", [P, DO, NH, HD], BF16, kind="ExternalInput")
    wk_d = nc.dram_tensor("wk", [P, DO, NH, HD], BF16, kind="ExternalInput")
    wv_d = nc.dram_tensor("wv", [P, DO, NH * HD], BF16, kind="ExternalInput")
    wo_d = nc.dram_tensor("wo", [P, NH, D], BF16, kind="ExternalInput")
    cs2_d = nc.dram_tensor("cs2", [P, S], F32, kind="ExternalInput")
    sn2_d = nc.dram_tensor("sn2", [P, S], F32, kind="ExternalInput")
    mka_d = nc.dram_tensor("mka", [P, 2 * SC], BF16, kind="ExternalInput")
    mkb_d = nc.dram_tensor("mkb", [P, 2 * SC], BF16, kind="ExternalInput")
    # 128 rows per chunk after ReduceScatter
    out_d = nc.dram_tensor("out", [QC * P, D], BF16, kind="ExternalOutput")

    tanh_scale = 1.0 / (SCAP * math.sqrt(HD))
    MUL = mybir.AluOpType.mult
    ADDOP = mybir.AluOpType.add
    TANH = mybir.ActivationFunctionType.Tanh
    EXP = mybir.ActivationFunctionType.Exp

    with tile.TileContext(nc) as tc:
        with (
            tc.tile_pool(name="pers", bufs=1) as pers,
            tc.tile_pool(name="xtp", bufs=2) as xtp,
            tc.tile_pool(name="etp", bufs=3) as etp,
            tc.tile_pool(name="dvp", bufs=1) as dvp,
            tc.tile_pool(name="rbp", bufs=1) as rbp,
            tc.tile_pool(name="atp", bufs=2) as atp,
            tc.tile_pool(name="otp", bufs=3) as otp,
            tc.tile_pool(name="psc", bufs=2, space="PSUM") as psc,
            tc.tile_pool(name="pso", bufs=1, space="PSUM") as pso,
            tc.tile_pool(name="ppj", bufs=1, space="PSUM") as ppj,
            tc.tile_pool(name="pop", bufs=1, space="PSUM") as pop,
            tc.tile_pool(name="dram", bufs=1, space="DRAM") as dpool,
        ):
            # ---- persistent SBUF ----
            QT = pers.tile([P, NH, S], BF16, tag="QT")
            KT = pers.tile([P, NH, S], BF16, tag="KT")
            V = pers.tile([P, ST, NH * HD], BF16, tag="V")
            wq_sb = pers.tile([P, DO, NH, HD], BF16, tag="wq")
            wk_sb = pers.tile([P, DO, NH, HD], BF16, tag="wk")
            wv_lo = pers.tile([P, DO // 2, NH * HD], BF16, tag="wv_lo")
            wv_hi = pers.tile([P, DO // 2, NH * HD], BF16, tag="wv_hi")
            wo_sb = pers.tile([P, NH, D], BF16, tag="wo")
            cs2_sb = pers.tile([P, S], F32, tag="cs2")
            sn2_sb = pers.tile([P, S], F32, tag="sn2")
            mka_sb = pers.tile([P, 2 * SC], BF16, tag="mka")
            mkb_sb = pers.tile([P, 2 * SC], BF16, tag="mkb")
            ones = pers.tile([P, P], BF16, tag="ones")

            nc.vector.memset(ones[:], 1.0)
            # two HWDGE queues; interleave so first P(0) units are fed early
            xt_c = {}

            HVOL = DO // 2 * SC

            def load_xt(t):
                x = xtp.tile([P, DO, SC], BF16, tag="xt", name=f"xt{t}")
                nc.sync.dma_start(
                    x[:].rearrange("p a b -> p (a b)"), xt_d[t])
                xt_c[t] = x

            nc.scalar.dma_start(wv_lo[:], wv_d[:, 0:DO // 2, :])
            x0 = xtp.tile([P, DO, SC], BF16, tag="xt", name="xt0")
            nc.sync.dma_start(
                x0[:].rearrange("p a b -> p (a b)")[:, 0:HVOL],
                xt_d[0][:, 0:HVOL])
            nc.scalar.dma_start(wv_hi[:], wv_d[:, DO // 2:DO, :])
            nc.sync.dma_start(
                x0[:].rearrange("p a b -> p (a b)")[:, HVOL:2 * HVOL],
                xt_d[0][:, HVOL:2 * HVOL])
            xt_c[0] = x0
            nc.scalar.dma_start(cs2_sb[:], cs2_d[:])
            nc.sync.dma_start(wq_sb[:], wq_d[:])
            nc.scalar.dma_start(sn2_sb[:], sn2_d[:])
            nc.sync.dma_start(wk_sb[:], wk_d[:])
            nc.scalar.dma_start(mka_sb[:], mka_d[:])
            nc.scalar.dma_start(mkb_sb[:], mkb_d[:])
            nc.scalar.dma_start(wo_sb[:], wo_d[:])
            load_xt(1)

            pm = [dpool.tile([SC, D], BF16, tag=f"pm{t}", name=f"pm{t}")
                  for t in range(QC)]
            rs = {}
            for t in range(QC):
                nseg = SPLITS[t]
                segr = SC // nseg          # input rows per segment
                for i in range(nseg):
                    rs[(t, i)] = dpool.tile(
                        [segr // 4, D], BF16, tag=f"rs{t}_{i}",
                        name=f"rs{t}_{i}")

            # ---------------- unit builders ----------------
            state = {"last_pm": None}

            def unit_V(t, stl):
                def f(t=t, stl=stl):
                    ps = ppj.tile([P, SC], F32, tag="ppj")
                    xc = xt_c[t]
                    for do in range(DO):
                        xs = xc[:, do, :]
                        wvh = wv_lo if do < DO // 2 else wv_hi
                        wvs = wvh[:, do % (DO // 2), :]
                        nc.tensor.matmul(
                            ps[:],
                            lhsT=xs[:, stl * P:(stl + 1) * P],
                            rhs=wvs,
                            start=(do == 0), stop=(do == DO - 1))
                    nc.scalar.copy(V[:, 4 * t + stl, :], ps[:])
                return f

            def unit_QK(t, h, which):
                def f(t=t, h=h, which=which):
                    w_sb = wq_sb if which == 0 else wk_sb
                    dst = QT if which == 0 else KT
                    ps = ppj.tile([P, SC], F32, tag="ppj")
                    xc = xt_c[t]
                    for do in range(DO):
                        xs = xc[:, do, :]
                        nc.tensor.matmul(
                            ps[:], lhsT=w_sb[:, do, h, :],
                            rhs=xs,
                            start=(do == 0), stop=(do == DO - 1))
                    sl = slice(t * SC, (t + 1) * SC)
                    tcos = dvp.tile([P, SC], F32, tag="tcos")
                    tsin = dvp.tile([P, SC], F32, tag="tsin")
                    nc.vector.tensor_tensor(
                        tcos[:], ps[:], cs2_sb[:, sl], MUL)
                    nc.vector.tensor_tensor(
                        tsin[0:HALF, :], ps[HALF:P, :],
                        sn2_sb[0:HALF, sl], MUL)
                    nc.vector.tensor_tensor(
                        tsin[HALF:P, :], ps[0:HALF, :],
                        sn2_sb[HALF:P, sl], MUL)
                    nc.vector.tensor_tensor(
                        dst[:, h, sl], tcos[:], tsin[:], ADDOP)
                return f

            def P_units(t):
                # V units first: they only need wv + the xt chunk, so the
                # PE can start before wq/wk finish loading.
                units = [unit_V(t, stl) for stl in range(4)]
                for h in range(NH):
                    units.append(unit_QK(t, h, 0))
                    units.append(unit_QK(t, h, 1))
                return units

            # ---- attention ----
            def consume(t, h, st):
                """Mask, then AV + row-sum matmuls for the previous group."""
                g, et = st.pop("prev")
                nkc = 4 * (t + 1)
                if 2 * g >= 4 * t:  # diagonal pair of k-tiles
                    mk = mka_sb if (2 * g - 4 * t) == 0 else mkb_sb
                    nc.vector.tensor_tensor(et[:], et[:], mk[:], MUL)
                if st.get("ps_o") is None:
                    st["ps_o"] = pso.tile([P, SC], F32, tag="pso",
                                          name=f"pso{t}_{h}")
                    st["ps_sum"] = pso.tile([P, SC], F32, tag="ps_sum",
                                            name=f"pss{t}_{h}")
                ps_o = st["ps_o"]
                ps_sum = st["ps_sum"]
                for j in (0, 1):
                    kc = 2 * g + j
                    nc.tensor.matmul(
                        ps_o[:],
                        lhsT=V[:, kc, h * HD:(h + 1) * HD],
                        rhs=et[:, j * SC:(j + 1) * SC],
                        start=(kc == 0), stop=(kc == nkc - 1))
                for j in (0, 1):
                    kc = 2 * g + j
                    nc.tensor.matmul(
                        ps_sum[:], lhsT=ones[:],
                        rhs=et[:, j * SC:(j + 1) * SC],
                        start=(kc == 0), stop=(kc == nkc - 1))

            def unit_A(t, h, g, st, cst):
                def f(t=t, h=h, g=g, st=st, cst=cst):
                    if cst.get("attnT") is None:
                        cst["attnT"] = atp.tile(
                            [P, NH, SC], BF16, tag="attnT",
                            name=f"attnT{t}")
                    psg = psc.tile([P, 2 * SC], F32, tag="score")
                    for j in (0, 1):
                        kc = 2 * g + j
                        nc.tensor.matmul(
                            psg[:, j * SC:(j + 1) * SC],
                            lhsT=KT[:, h, kc * P:(kc + 1) * P],
                            rhs=QT[:, h, t * SC:(t + 1) * SC],
                            start=True, stop=True)
                    nc.scalar.activation(psg[:], psg[:], TANH,
                                         scale=tanh_scale)
                    et = etp.tile([P, 2 * SC], BF16, tag="et")
                    nc.scalar.activation(et[:], psg[:], EXP, scale=SCAP)
                    if "prev" in st:
                        consume(t, h, st)
                    st["prev"] = (g, et)
                return f

            def unit_Afin(t, h, st, cst):
                def f(t=t, h=h, st=st, cst=cst):
                    consume(t, h, st)
                    rb = rbp.tile([P, SC], F32, tag="rb")
                    nc.vector.reciprocal_approx_fast(
                        out=rb[:], in_=st["ps_sum"][:])
                    nc.vector.tensor_tensor(
                        cst["attnT"][:, h, :], st["ps_o"], rb[:], MUL)
                return f

            def A_units(t, cst):
                units = []
                for h in range(NH):
                    st = {}
                    for g in range(2 * (t + 1)):
                        units.append(unit_A(t, h, g, st, cst))
                    units.append(unit_Afin(t, h, st, cst))
                return units

            def unit_O(t, stl, cst):
                def f(t=t, stl=stl, cst=cst):
                    attnT = cst["attnT"]
                    for mc in range(4):
                        if t == 3 and mc % 2 == 1:
                            psbig = psc.tile([P, 2 * SC], F32, tag="score")
                            ps = psbig[:, 0:SC]
                        else:
                            ps = pop.tile([P, SC], F32, tag="pop")
                        for h in range(NH):
                            nc.tensor.matmul(
                                ps,
                                lhsT=attnT[:, h, stl * P:(stl + 1) * P],
                                rhs=wo_sb[:, h, mc * SC:(mc + 1) * SC],
                                start=(h == 0), stop=(h == NH - 1))
                        ot = otp.tile([P, SC], BF16, tag="ot")
                        nc.vector.tensor_copy(ot[:], ps)
                        state["last_pm"] = nc.sync.dma_start(
                            pm[t][stl * P:(stl + 1) * P,
                                  mc * SC:(mc + 1) * SC], ot[:])
                    nseg = SPLITS[t]
                    slabs_per_seg = 4 // nseg
                    if (stl + 1) % slabs_per_seg == 0:
                        i = stl // slabs_per_seg
                        segr = SC // nseg
                        nc.gpsimd.collective_compute(
                            "ReduceScatter", ADDOP,
                            replica_groups=GROUPS,
                            ins=[pm[t][i * segr:(i + 1) * segr, :].opt()],
                            outs=[rs[(t, i)].opt()],
                        )
                return f

            def O_units(t, cst):
                return [unit_O(t, stl, cst) for stl in range(4)]

            # ---------------- emission pipeline ----------------
            load_xt(2)
            load_xt(3)
            cst = {t: {} for t in range(QC)}
            for u in P_units(0):
                u()
            for u in _weave(A_units(0, cst[0]), P_units(1)):
                u()
            for u in _weave(A_units(1, cst[1]), P_units(2),
                            O_units(0, cst[0])):
                u()
            for u in _weave(A_units(2, cst[2]), P_units(3),
                            O_units(1, cst[1])):
                u()
            for u in _weave(A_units(3, cst[3]), O_units(2, cst[2])):
                u()
            for u in O_units(3, cst[3]):
                u()
            # rs -> out copies at the very end: their RS-completion waits
            # must not head-of-line-block the pm-write DMA queue, so pin
            # them after the last pm write with an explicit dependency.
            for t in range(QC):
                nseg = SPLITS[t]
                segr = SC // nseg
                for i in range(nseg):
                    r0 = t * P + i * (segr // 4)
                    od = nc.sync.dma_start(
                        out_d[r0:r0 + segr // 4, :], rs[(t, i)][:])
                    add_dep_helper(od.ins, state["last_pm"].ins, sync=False,
                                   reason="out copies after all pm writes")

    nc.compile()
    return nc


def _prep_core_inputs(inputs, w_in, w_out, rope_sin, rope_cos):
    """Build the 8 per-core input maps (numpy, pre-transposed, bf16)."""
    bf = ml_dtypes.bfloat16
    cs2 = np.concatenate([rope_cos.T, rope_cos.T], axis=0).astype(np.float32)
    sn2 = np.concatenate([-rope_sin.T, rope_sin.T], axis=0).astype(np.float32)
    cs2 = np.ascontiguousarray(cs2)
    sn2 = np.ascontiguousarray(sn2)
    # masks for diagonal k-tile pairs: mask_j[p, q] = (q >= 128*j + p)
    q = np.arange(SC)[None, :]
    p = np.arange(P)[:, None]
    masks = [(q >= 128 * j + p).astype(bf) for j in range(4)]
    mka = np.ascontiguousarray(np.concatenate([masks[0], masks[1]], axis=1))
    mkb = np.ascontiguousarray(np.concatenate([masks[2], masks[3]], axis=1))
    in_maps = []
    for c in range(NCORES):
        g, pos = c // 4, c % 4
        hsel = slice(4 * pos, 4 * pos + 4)
        xt = np.ascontiguousarray(
            inputs[g].T.reshape(DO, P, QC, SC).transpose(2, 1, 0, 3)
            .reshape(QC, P, DO * SC)).astype(bf)
        wq = np.ascontiguousarray(
            w_in[:, hsel, 0:HD].reshape(DO, P, NH, HD).transpose(1, 0, 2, 3)
        ).astype(bf)
        wk = np.ascontiguousarray(
            w_in[:, hsel, HD:2 * HD].reshape(DO, P, NH, HD)
            .transpose(1, 0, 2, 3)).astype(bf)
        wv = np.ascontiguousarray(
            w_in[:, hsel, 2 * HD:3 * HD].reshape(DO, P, NH, HD)
            .transpose(1, 0, 2, 3).reshape(P, DO, NH * HD)).astype(bf)
        wo = np.ascontiguousarray(
            w_out[hsel].transpose(1, 0, 2)).astype(bf)
        in_maps.append({"xt": xt, "wq": wq, "wk": wk, "wv": wv, "wo": wo,
                       "cs2": cs2, "sn2": sn2, "mka": mka, "mkb": mkb})
    return in_maps


def kernel(inputs, w_in, w_out, rope_sin, rope_cos, mask=None):
    global LAST_EXEC_NS, LAST_RESULT
    inputs = np.asarray(inputs, dtype=np.float32)
    w_in = np.asarray(w_in, dtype=np.float32)
    w_out = np.asarray(w_out, dtype=np.float32)
    rope_sin = np.asarray(rope_sin, dtype=np.float32)
    rope_cos = np.asarray(rope_cos, dtype=np.float32)

    if "nc" not in _CACHE:
        _CACHE["nc"] = build_nc()
    nc = _CACHE["nc"]

    in_maps = _prep_core_inputs(inputs, w_in, w_out, rope_sin, rope_cos)
    trace = bool(int(os.environ.get("BASS_PROFILE", "0")))
    if trace:
        _install_profile_shim()
    tmpdir = os.environ.get("BASS_TRACE_DIR") or None
    try:
        res = bass_utils.run_bass_kernel_spmd(
            nc, in_maps, core_ids=list(range(NCORES)), trace=trace,
            tmpdir=tmpdir)
    except Exception:
        if not trace:
            raise
        res = bass_utils.run_bass_kernel_spmd(
            nc, in_maps, core_ids=list(range(NCORES)), trace=False)
    LAST_EXEC_NS = res.exec_time_ns
    LAST_RESULT = res

    out = np.empty((B, S, D), dtype=np.float32)
    for c in range(NCORES):
        g, pos = c // 4, c % 4
        o = res.results[c]["out"]  # [QC*P, D] bf16
        of = np.asarray(o).astype(np.float32)
        for t in range(QC):
            nseg = SPLITS[t]
            segr = SC // nseg
            seg4 = segr // 4
            for i in range(nseg):
                r0 = t * P + i * seg4
                q0 = t * SC + i * segr + pos * seg4
                out[g, q0:q0 + seg4, :] = of[r0:r0 + seg4, :]
    return out

